# revision 6
# baseline (speedup 1.0000x reference)
"""Distributed Trainium2 kernel for nn_Attention_9740985827390.

Sharding: heads across 8 cores (2 heads/core, both batches local).
Dataflow is fully "transposed" (feature-major) so every matmul contracts
over partitions with zero on-device input transposes:
  - host passes xT (DIM, B*N) bf16
  - QT/KT = W.T @ xT  (d-major),  V via PE-transpose of VT tiles (j-major)
  - simT[j,i] accumulated per (h, i-chunk) over j-tiles; causal tiles skipped
  - bias is host-transposed, causal-masked, bf16; key-pad mask folded into
    the exp() per-partition bias operand
  - LayerNorm: rsig cancels inside l2norm(q); only the mean matters and it
    is applied as a rank-1 PSUM-accumulated correction (-colsum(WqG)/DIM x mu)
  - softmax denominator: ones-column appended to V, divided in the epilogue
  - out = sum_h attnout_h @ Wo[rows_h]: row-parallel partials summed on host
"""

import numpy as np
import ml_dtypes

import concourse.bass as bass
import concourse.mybir as mybir
import concourse.tile as tile
from concourse import bacc
from concourse.bass_utils import run_bass_kernel_spmd
from concourse.masks import make_identity

B, N, DIM = 2, 2048, 1024
H, DH = 16, 64
NNK = 2
SCALE = 8.0
NCORE = 8
HPC = H // NCORE          # heads per core = 2
NEG = -1e30
ICW = 512                 # i-chunk width
NIC = N // ICW            # 4 i-chunks per batch
JTW = 128                 # j-tile width
NJT = N // JTW            # 16 j-tiles per batch
NDT = DIM // 128          # 8 dim tiles
HW = DH + 1               # 65: head block width in vbuf (v cols + ones col)
VSTRIDE = HPC * HW        # 130: per-j-tile column block in vbuf

BF16 = mybir.dt.bfloat16
F32 = mybir.dt.float32
AF = mybir.ActivationFunctionType
ALU = mybir.AluOpType

bf = ml_dtypes.bfloat16

_CACHE = {}


def build_nc():
    nc = bacc.Bacc("TRN2", target_bir_lowering=False, debug=False,
                   enable_asserts=False, num_devices=NCORE)
    TOK = B * N
    xT_d = nc.declare_dram_parameter("xT", [DIM, TOK], BF16, isOutput=False)
    wq_d = nc.declare_dram_parameter("wq", [128, NDT * 128], BF16, isOutput=False)
    wk_d = nc.declare_dram_parameter("wk", [128, NDT * 128], BF16, isOutput=False)
    wv_d = nc.declare_dram_parameter("wv", [128, NDT * 128], BF16, isOutput=False)
    wo_d = nc.declare_dram_parameter("wo", [128, DIM], BF16, isOutput=False)
    sqn_d = nc.declare_dram_parameter("sqn", [1, 128], BF16, isOutput=False)
    hr_d = nc.declare_dram_parameter("hr", [4, 128], BF16, isOutput=False)
    nkT_d = nc.declare_dram_parameter("nkT", [128, NNK], BF16, isOutput=False)
    nvA_d = nc.declare_dram_parameter("nvA", [NNK, VSTRIDE], BF16, isOutput=False)
    km_d = nc.declare_dram_parameter("km", [128, B * NJT], F32, isOutput=False)
    biasT_d = nc.declare_dram_parameter("biasT", [HPC, N, N], BF16, isOutput=False)
    out_d = nc.declare_dram_parameter("out", [B, N, DIM], BF16, isOutput=True)

    with tile.TileContext(nc) as tc:
        with tc.tile_pool(name="persist", bufs=1) as pp, \
             tc.tile_pool(name="work", bufs=3) as wp, \
             tc.tile_pool(name="bias", bufs=6) as bp, \
             tc.tile_pool(name="psB", bufs=3, space="PSUM") as psB, \
             tc.tile_pool(name="psS", bufs=2, space="PSUM") as psS, \
             tc.tile_pool(name="poP", bufs=1, space="PSUM") as poP:

            # ---------------- persistent SBUF ----------------
            xts = []
            for dt in range(NDT):
                t = pp.tile([128, TOK], BF16, tag=f"xt{dt}", name=f"xt{dt}")
                nc.sync.dma_start(out=t[:, :], in_=xT_d[dt * 128:(dt + 1) * 128, :])
                xts.append(t)
            wq_s = pp.tile([128, NDT * 128], BF16, tag="wq")
            nc.sync.dma_start(out=wq_s[:, :], in_=wq_d[:, :])
            wk_s = pp.tile([128, NDT * 128], BF16, tag="wk")
            nc.sync.dma_start(out=wk_s[:, :], in_=wk_d[:, :])
            wv_s = pp.tile([128, NDT * 128], BF16, tag="wv")
            nc.sync.dma_start(out=wv_s[:, :], in_=wv_d[:, :])
            wo_s = pp.tile([128, DIM], BF16, tag="wo")
            nc.sync.dma_start(out=wo_s[:, :], in_=wo_d[:, :])
            sqn_s = pp.tile([1, 128], BF16, tag="sqn")
            nc.sync.dma_start(out=sqn_s[:, :], in_=sqn_d[:, :])
            hrq_s = pp.tile([2, 128], BF16, tag="hrq")
            nc.sync.dma_start(out=hrq_s[:, :], in_=hr_d[0:2, :])
            hrk_s = pp.tile([2, 128], BF16, tag="hrk")
            nc.sync.dma_start(out=hrk_s[:, :], in_=hr_d[2:4, :])
            nkT_s = pp.tile([128, NNK], BF16, tag="nkT")
            nc.sync.dma_start(out=nkT_s[:, :], in_=nkT_d[:, :])
            nvA_s = pp.tile([NNK, VSTRIDE], BF16, tag="nvA")
            nc.sync.dma_start(out=nvA_s[:, :], in_=nvA_d[:, :])
            km_s = pp.tile([128, B * NJT], F32, tag="km")
            nc.sync.dma_start(out=km_s[:, :], in_=km_d[:, :])

            ident = pp.tile([128, 128], BF16, tag="ident")
            make_identity(nc, ident[:, :])
            ones_col = pp.tile([128, 1], BF16, tag="ones_col")
            nc.vector.memset(ones_col[:, :], 1.0)
            ones_row = pp.tile([1, 128], BF16, tag="ones_row")
            nc.vector.memset(ones_row[:, :], 1.0)
            hsel = pp.tile([128, HPC], BF16, tag="hsel")
            nc.vector.memset(hsel[:, :], 0.0)
            for h in range(HPC):
                nc.vector.memset(hsel[h * DH:(h + 1) * DH, h:h + 1], 1.0)

            qhat = [pp.tile([128, N], BF16, tag=f"qhat{b}", name=f"qhat{b}") for b in range(B)]
            khat = [pp.tile([128, N], BF16, tag=f"khat{b}", name=f"khat{b}") for b in range(B)]
            nkhat = pp.tile([128, NNK], BF16, tag="nkhat")
            vbuf = [pp.tile([128, NJT * VSTRIDE], BF16, tag=f"vbuf{b}", name=f"vbuf{b}") for b in range(B)]
            outT = [pp.tile([128, N], BF16, tag=f"outT{b}", name=f"outT{b}") for b in range(B)]

            # ones columns in vbuf (col DH of each head block of each j-tile)
            for b in range(B):
                for jt in range(NJT):
                    for h in range(HPC):
                        col = jt * VSTRIDE + h * HW + DH
                        nc.vector.memset(vbuf[b][:, col:col + 1], 1.0)

            # ---------------- projections ----------------
            def proj_psum(w_s, col0, close):
                ps = psB.tile([128, ICW], F32, tag="big")
                for dt in range(NDT):
                    nc.tensor.matmul(
                        ps[:, :],
                        lhsT=w_s[:, dt * 128:(dt + 1) * 128],
                        rhs=xts[dt][:, col0:col0 + ICW],
                        start=(dt == 0), stop=(close and dt == NDT - 1))
                return ps

            def norm_finish(ps_raw, w, hr_row, sqrt_scale, out_ap):
                # raw (128,w) f32 psum -> l2-normalized bf16 in out_ap
                raw_sb = wp.tile([128, ICW], BF16, tag="rawsb")
                nc.scalar.activation(raw_sb[:, :w], ps_raw[:, :w], AF.Copy)
                sq_sb = wp.tile([128, ICW], BF16, tag="sqsb")
                nc.scalar.activation(sq_sb[:, :w], ps_raw[:, :w], AF.Square)
                ps_ss = psS.tile([HPC, ICW], F32, tag="small")
                nc.tensor.matmul(ps_ss[:, :w], lhsT=hsel[:, :], rhs=sq_sb[:, :w],
                                 start=True, stop=True)
                rec = wp.tile([HPC, ICW], F32, tag="rec")
                nc.vector.reciprocal(rec[:, :w], ps_ss[:, :w])
                rno = wp.tile([HPC, ICW], BF16, tag="rno")
                nc.scalar.activation(rno[:, :w], rec[:, :w], AF.Sqrt,
                                     scale=float(sqrt_scale))
                ps_rep = psS.tile([128, ICW], F32, tag="small")
                nc.tensor.matmul(ps_rep[:, :w], lhsT=hr_row[:, :],
                                 rhs=rno[:, :w], start=True, stop=True)
                nc.vector.tensor_tensor(out_ap, raw_sb[:, :w], ps_rep[:, :w],
                                        op=ALU.mult)

            for b in range(B):
                for c in range(NIC):
                    col0 = b * N + c * ICW
                    # sum(x) row for the mean correction
                    ps_mu = psS.tile([1, ICW], F32, tag="small")
                    for dt in range(NDT):
                        nc.tensor.matmul(ps_mu[:, :], lhsT=ones_col[:, :],
                                         rhs=xts[dt][:, col0:col0 + ICW],
                                         start=(dt == 0), stop=(dt == NDT - 1))
                    mu_row = wp.tile([1, ICW], BF16, tag="murow")
                    nc.scalar.activation(mu_row[:, :], ps_mu[:, :], AF.Copy)
                    # Q: raw projection + rank-1 mean correction
                    ps_q = proj_psum(wq_s, col0, close=False)
                    nc.tensor.matmul(ps_q[:, :], lhsT=sqn_s[:, :], rhs=mu_row[:, :],
                                     start=False, stop=True)
                    norm_finish(ps_q, ICW, hrq_s, SCALE * SCALE,
                                qhat[b][:, c * ICW:(c + 1) * ICW])
                    # K
                    ps_k = proj_psum(wk_s, col0, close=True)
                    norm_finish(ps_k, ICW, hrk_s, 1.0,
                                khat[b][:, c * ICW:(c + 1) * ICW])
                    # V: project then PE-transpose into j-major vbuf
                    ps_v = proj_psum(wv_s, col0, close=True)
                    vT_sb = wp.tile([128, ICW], BF16, tag="vtsb")
                    nc.scalar.activation(vT_sb[:, :], ps_v[:, :], AF.Copy)
                    for tt in range(ICW // 128):
                        jt = c * (ICW // 128) + tt
                        ps_t = psS.tile([128, 128], BF16, tag="small")
                        nc.tensor.transpose(ps_t[:, :], vT_sb[:, tt * 128:(tt + 1) * 128],
                                            ident[:, :])
                        for h in range(HPC):
                            nc.vector.tensor_copy(
                                vbuf[b][:, jt * VSTRIDE + h * HW:jt * VSTRIDE + h * HW + DH],
                                ps_t[:, h * DH:(h + 1) * DH])

            # null-k normalization (shared across b)
            nksq = wp.tile([128, NNK], BF16, tag="nksq")
            nc.scalar.activation(nksq[:, :], nkT_s[:, :], AF.Square)
            ps_nss = psS.tile([HPC, NNK], F32, tag="small")
            nc.tensor.matmul(ps_nss[:, :], lhsT=hsel[:, :], rhs=nksq[:, :],
                             start=True, stop=True)
            nrec = wp.tile([HPC, NNK], F32, tag="nrec")
            nc.vector.reciprocal(nrec[:, :], ps_nss[:, :])
            nrno = wp.tile([HPC, NNK], BF16, tag="nrno")
            nc.scalar.activation(nrno[:, :], nrec[:, :], AF.Sqrt, scale=1.0)
            ps_nrep = psS.tile([128, NNK], F32, tag="small")
            nc.tensor.matmul(ps_nrep[:, :], lhsT=hrk_s[:, :], rhs=nrno[:, :],
                             start=True, stop=True)
            nc.vector.tensor_tensor(nkhat[:, :], nkT_s[:, :], ps_nrep[:, :], op=ALU.mult)

            # ---------------- attention ----------------
            for h in range(HPC):
                hp = h * DH
                for ic in range(NIC):
                    i0 = ic * ICW
                    pos = [poP.tile([HW, ICW], F32, tag=f"po{b}", name=f"po{b}") for b in range(B)]
                    njts = (ic + 1) * (ICW // JTW)
                    for jt in range(njts):
                        bt = bp.tile([128, ICW], BF16, tag="bias")
                        nc.sync.dma_start(
                            out=bt[:, :],
                            in_=biasT_d[h, jt * JTW:(jt + 1) * JTW, i0:i0 + ICW])
                        for b in range(B):
                            ps_s = psB.tile([128, ICW], F32, tag="big")
                            nc.tensor.matmul(
                                ps_s[:, :],
                                lhsT=khat[b][hp:hp + DH, jt * JTW:(jt + 1) * JTW],
                                rhs=qhat[b][hp:hp + DH, i0:i0 + ICW],
                                start=True, stop=True, skip_group_check=True)
                            s_sb = wp.tile([128, ICW], F32, tag="ssb")
                            nc.vector.tensor_tensor(s_sb[:, :], ps_s[:, :], bt[:, :],
                                                    op=ALU.add)
                            e_sb = wp.tile([128, ICW], BF16, tag="esb")
                            nc.scalar.activation(
                                e_sb[:, :], s_sb[:, :], AF.Exp,
                                bias=km_s[:, b * NJT + jt:b * NJT + jt + 1])
                            nc.tensor.matmul(
                                pos[b][:, :],
                                lhsT=vbuf[b][:, jt * VSTRIDE + h * HW:(jt * VSTRIDE + h * HW) + HW],
                                rhs=e_sb[:, :],
                                start=(jt == 0), stop=False, skip_group_check=True)
                    # null kv tile
                    for b in range(B):
                        ps_n = psS.tile([NNK, ICW], F32, tag="small")
                        nc.tensor.matmul(ps_n[:, :],
                                         lhsT=nkhat[hp:hp + DH, :],
                                         rhs=qhat[b][hp:hp + DH, i0:i0 + ICW],
                                         start=True, stop=True, skip_group_check=True)
                        en = wp.tile([NNK, ICW], BF16, tag="en")
                        nc.scalar.activation(en[:, :], ps_n[:, :], AF.Exp)
                        nc.tensor.matmul(pos[b][:, :],
                                         lhsT=nvA_s[:, h * HW:(h + 1) * HW],
                                         rhs=en[:, :],
                                         start=False, stop=True, skip_group_check=True)
                    # epilogue: divide by the denominator row
                    for b in range(B):
                        rd32 = wp.tile([1, ICW], F32, tag="rd32")
                        nc.vector.reciprocal(rd32[:, :], pos[b][DH:DH + 1, :])
                        rdb = wp.tile([1, ICW], BF16, tag="rdb")
                        nc.scalar.activation(rdb[:, :], rd32[:, :], AF.Copy)
                        ps_r = psS.tile([DH, ICW], F32, tag="small")
                        nc.tensor.matmul(ps_r[:, :], lhsT=ones_row[:, :DH], rhs=rdb[:, :],
                                         start=True, stop=True, skip_group_check=True)
                        oc_sb = wp.tile([DH, ICW], BF16, tag="ocsb")
                        nc.scalar.activation(oc_sb[:, :], pos[b][:DH, :], AF.Copy)
                        nc.vector.tensor_tensor(
                            outT[b][hp:hp + DH, i0:i0 + ICW],
                            oc_sb[:, :], ps_r[:, :], op=ALU.mult)

            # -------- output projection (row-parallel partial, host-summed) --------
            for b in range(B):
                for tt in range(N // 128):
                    for cc in range(DIM // ICW):
                        ps_f = psB.tile([128, ICW], F32, tag="big")
                        nc.tensor.matmul(ps_f[:, :],
                                         lhsT=outT[b][:, tt * 128:(tt + 1) * 128],
                                         rhs=wo_s[:, cc * ICW:(cc + 1) * ICW],
                                         start=True, stop=True)
                        f_sb = wp.tile([128, ICW], BF16, tag="fsb")
                        nc.any.tensor_copy(f_sb[:, :], ps_f[:, :])
                        nc.sync.dma_start(
                            out=out_d[b, tt * 128:(tt + 1) * 128, cc * ICW:(cc + 1) * ICW],
                            in_=f_sb[:, :])
    nc.compile()
    return nc


def prep_inputs(x, mask, attn_bias, gamma, null_kv, Wq, Wkv, q_scale, k_scale, Wo):
    """Host-side sharding + layout prep. Returns in_maps for 8 cores."""
    x = np.asarray(x, np.float32)
    mask = np.asarray(mask, bool)
    attn_bias = np.asarray(attn_bias, np.float32)
    gamma = np.asarray(gamma, np.float32)
    null_kv = np.asarray(null_kv, np.float32)
    Wq = np.asarray(Wq, np.float32)
    Wkv = np.asarray(Wkv, np.float32)
    q_scale = np.asarray(q_scale, np.float32)
    k_scale = np.asarray(k_scale, np.float32)
    Wo = np.asarray(Wo, np.float32)

    TOK = B * N
    xT = np.ascontiguousarray(x.reshape(TOK, DIM).T).astype(bf)
    WqG = Wq * gamma[:, None]
    qsks = (q_scale * k_scale).astype(np.float32)
    trimask = np.tril(np.ones((N, N), dtype=bool), -1)  # [j, i] True where j > i
    kmf = np.where(mask, 0.0, NEG).astype(np.float32)   # (B, N)
    km = np.ascontiguousarray(
        kmf.reshape(B, NJT, JTW).transpose(2, 0, 1).reshape(JTW, B * NJT))

    def wlayout(w):  # (1024, 128) -> (128, 8*128) device layout
        return np.ascontiguousarray(
            w.reshape(NDT, 128, 128).transpose(1, 0, 2).reshape(128, NDT * 128)
        ).astype(bf)

    in_maps = []
    for c in range(NCORE):
        h0 = HPC * c
        wq_c = WqG[:, h0 * DH:(h0 + HPC) * DH]
        wk_c = Wkv[:, h0 * DH:(h0 + HPC) * DH]
        wv_c = Wkv[:, H * DH + h0 * DH:H * DH + (h0 + HPC) * DH]
        sqn = (-wq_c.sum(axis=0, dtype=np.float64) / DIM).astype(np.float32)[None, :]
        hr = np.zeros((4, 128), np.float32)
        hr[0, :DH] = 1.0
        hr[1, DH:] = 1.0
        hr[2, :DH] = qsks
        hr[3, DH:] = qsks
        nkv = null_kv[h0:h0 + HPC].reshape(HPC, NNK, 2, DH)
        nk = nkv[:, :, 0]   # (HPC, NNK, DH)
        nv = nkv[:, :, 1]
        nkT = np.ascontiguousarray(nk.transpose(0, 2, 1).reshape(HPC * DH, NNK))
        nvA = np.zeros((NNK, VSTRIDE), np.float32)
        for h in range(HPC):
            nvA[:, h * HW:h * HW + DH] = nv[h]
            nvA[:, h * HW + DH] = 1.0
        bT = np.ascontiguousarray(attn_bias[h0:h0 + HPC].transpose(0, 2, 1)).copy()
        bT[:, trimask] = NEG
        in_maps.append({
            "xT": xT,
            "wq": wlayout(wq_c), "wk": wlayout(wk_c), "wv": wlayout(wv_c),
            "wo": Wo[h0 * DH:(h0 + HPC) * DH, :].astype(bf),
            "sqn": sqn.astype(bf), "hr": hr.astype(bf),
            "nkT": nkT.astype(bf), "nvA": nvA.astype(bf),
            "km": km, "biasT": bT.astype(bf),
        })
    return in_maps


def kernel(x, mask, attn_bias, gamma, null_kv, Wq, Wkv, q_scale, k_scale, Wo):
    if "nc" not in _CACHE:
        _CACHE["nc"] = build_nc()
    nc = _CACHE["nc"]
    in_maps = prep_inputs(x, mask, attn_bias, gamma, null_kv, Wq, Wkv,
                          q_scale, k_scale, Wo)
    res = run_bass_kernel_spmd(nc, in_maps, core_ids=list(range(NCORE)))
    acc = np.zeros((B, N, DIM), np.float32)
    for c in range(NCORE):
        acc += np.asarray(res.results[c]["out"], dtype=np.float32)
    return acc


# revision 16
# speedup vs baseline: 1.0239x; 1.0239x over previous
"""Distributed Trainium2 kernel for nn_Attention_9740985827390.

Sharding: heads across 8 cores (2 heads/core, both batches local).
Dataflow is fully "transposed" (feature-major) so every matmul contracts
over partitions with zero on-device input transposes:
  - host passes xT (DIM, B*N) bf16
  - QT/KT = W.T @ xT  (d-major),  V via PE-transpose of VT tiles (j-major)
  - simT[j,i] accumulated per (h, i-chunk) over j-tiles; causal tiles skipped
  - bias is host-transposed, causal-masked, bf16; key-pad mask folded into
    the exp() per-partition bias operand
  - LayerNorm: rsig cancels inside l2norm(q); only the mean matters and it
    is applied as a rank-1 PSUM-accumulated correction (-colsum(WqG)/DIM x mu)
  - softmax denominator: ones-column appended to V, divided in the epilogue
  - out = sum_h attnout_h @ Wo[rows_h]: row-parallel partials summed on host
"""

import numpy as np
import ml_dtypes

import concourse.bass as bass
import concourse.mybir as mybir
import concourse.tile as tile
from concourse import bacc
from concourse.bass_utils import run_bass_kernel_spmd
from concourse.masks import make_identity

B, N, DIM = 2, 2048, 1024
H, DH = 16, 64
NNK = 2
SCALE = 8.0
NCORE = 8
HPC = H // NCORE          # heads per core = 2
NEG = -1e30
ICW = 512                 # i-chunk width
NIC = N // ICW            # 4 i-chunks per batch
JTW = 128                 # j-tile width
NJT = N // JTW            # 16 j-tiles per batch
NDT = DIM // 128          # 8 dim tiles
HW = DH + 1               # 65: head block width in vbuf (v cols + ones col)
VSTRIDE = HPC * HW        # 130: per-j-tile column block in vbuf

BF16 = mybir.dt.bfloat16
F32 = mybir.dt.float32
AF = mybir.ActivationFunctionType
ALU = mybir.AluOpType

bf = ml_dtypes.bfloat16

_CACHE = {}


def build_nc():
    nc = bacc.Bacc("TRN2", target_bir_lowering=False, debug=False,
                   enable_asserts=False, num_devices=NCORE)
    TOK = B * N
    xT_d = nc.declare_dram_parameter("xT", [DIM, TOK], BF16, isOutput=False)
    wq_d = nc.declare_dram_parameter("wq", [128, NDT * 128], BF16, isOutput=False)
    wk_d = nc.declare_dram_parameter("wk", [128, NDT * 128], BF16, isOutput=False)
    wv_d = nc.declare_dram_parameter("wv", [128, NDT * 128], BF16, isOutput=False)
    wo_d = nc.declare_dram_parameter("wo", [128, DIM], BF16, isOutput=False)
    sqn_d = nc.declare_dram_parameter("sqn", [1, 128], BF16, isOutput=False)
    hr_d = nc.declare_dram_parameter("hr", [4, 128], BF16, isOutput=False)
    nkT_d = nc.declare_dram_parameter("nkT", [128, NNK], BF16, isOutput=False)
    nvA_d = nc.declare_dram_parameter("nvA", [NNK, VSTRIDE], BF16, isOutput=False)
    km_d = nc.declare_dram_parameter("km", [128, B * NJT], F32, isOutput=False)
    biasT_d = nc.declare_dram_parameter("biasT", [HPC, N, N], BF16, isOutput=False)
    out_d = nc.declare_dram_parameter("out", [B, N, DIM], BF16, isOutput=True)

    with tile.TileContext(nc) as tc:
        import contextlib
        with tc.tile_pool(name="persist", bufs=1) as pp, \
             tc.tile_pool(name="work", bufs=3) as wp, \
             tc.tile_pool(name="bias", bufs=8) as bp:

            # ---------------- persistent SBUF ----------------
            xts = []
            for dt in range(NDT):
                t = pp.tile([128, TOK], BF16, tag=f"xt{dt}", name=f"xt{dt}")
                nc.sync.dma_start(out=t[:, :], in_=xT_d[dt * 128:(dt + 1) * 128, :])
                xts.append(t)
            wq_s = pp.tile([128, NDT * 128], BF16, tag="wq")
            nc.sync.dma_start(out=wq_s[:, :], in_=wq_d[:, :])
            wk_s = pp.tile([128, NDT * 128], BF16, tag="wk")
            nc.sync.dma_start(out=wk_s[:, :], in_=wk_d[:, :])
            wv_s = pp.tile([128, NDT * 128], BF16, tag="wv")
            nc.sync.dma_start(out=wv_s[:, :], in_=wv_d[:, :])
            wo_s = pp.tile([128, DIM], BF16, tag="wo")
            nc.sync.dma_start(out=wo_s[:, :], in_=wo_d[:, :])
            sqn_s = pp.tile([1, 128], BF16, tag="sqn")
            nc.sync.dma_start(out=sqn_s[:, :], in_=sqn_d[:, :])
            hrq_s = pp.tile([2, 128], BF16, tag="hrq")
            nc.sync.dma_start(out=hrq_s[:, :], in_=hr_d[0:2, :])
            hrk_s = pp.tile([2, 128], BF16, tag="hrk")
            nc.sync.dma_start(out=hrk_s[:, :], in_=hr_d[2:4, :])
            nkT_s = pp.tile([128, NNK], BF16, tag="nkT")
            nc.sync.dma_start(out=nkT_s[:, :], in_=nkT_d[:, :])
            nvA_s = pp.tile([NNK, VSTRIDE], BF16, tag="nvA")
            nc.sync.dma_start(out=nvA_s[:, :], in_=nvA_d[:, :])
            km_s = pp.tile([128, B * NJT], F32, tag="km")
            nc.sync.dma_start(out=km_s[:, :], in_=km_d[:, :])

            ident = pp.tile([128, 128], BF16, tag="ident")
            make_identity(nc, ident[:, :])
            ones_col = pp.tile([128, 1], BF16, tag="ones_col")
            nc.vector.memset(ones_col[:, :], 1.0)
            ones_row = pp.tile([1, 128], BF16, tag="ones_row")
            nc.vector.memset(ones_row[:, :], 1.0)
            hsel = pp.tile([128, HPC], BF16, tag="hsel")
            nc.vector.memset(hsel[:, :], 0.0)
            for h in range(HPC):
                nc.vector.memset(hsel[h * DH:(h + 1) * DH, h:h + 1], 1.0)

            qhat = [pp.tile([128, N], BF16, tag=f"qhat{b}", name=f"qhat{b}") for b in range(B)]
            khat = [pp.tile([128, N], BF16, tag=f"khat{b}", name=f"khat{b}") for b in range(B)]
            nkhat = pp.tile([128, NNK], BF16, tag="nkhat")
            vbuf = [pp.tile([128, NJT * VSTRIDE], BF16, tag=f"vbuf{b}", name=f"vbuf{b}") for b in range(B)]
            outT = [pp.tile([128, N], BF16, tag=f"outT{b}", name=f"outT{b}") for b in range(B)]

            # ones columns in vbuf (col DH of each head block of each j-tile)
            for b in range(B):
                for jt in range(NJT):
                    for h in range(HPC):
                        col = jt * VSTRIDE + h * HW + DH
                        nc.vector.memset(vbuf[b][:, col:col + 1], 1.0)

            # ---------------- shared PSUM pools (8 banks total) ----------------
            all_ps = contextlib.ExitStack()
            psB = all_ps.enter_context(tc.tile_pool(name="psB", bufs=3, space="PSUM"))
            psS = all_ps.enter_context(tc.tile_pool(name="psS", bufs=2, space="PSUM"))
            poP = all_ps.enter_context(tc.tile_pool(name="poA", bufs=2, space="PSUM"))
            psF = all_ps.enter_context(tc.tile_pool(name="psFa", bufs=1, space="PSUM"))

            # ---------------- projections ----------------

            def proj_psum(w_s, col0, close):
                ps = psB.tile([128, ICW], F32, tag="big")
                for dt in range(NDT):
                    nc.tensor.matmul(
                        ps[:, :],
                        lhsT=w_s[:, dt * 128:(dt + 1) * 128],
                        rhs=xts[dt][:, col0:col0 + ICW],
                        start=(dt == 0), stop=(close and dt == NDT - 1))
                return ps

            def norm_finish(ps_raw, w, hr_row, sqrt_scale, out_ap):
                # raw (128,w) f32 psum -> l2-normalized bf16 in out_ap
                raw_sb = wp.tile([128, ICW], BF16, tag="rawsb", bufs=6)
                nc.scalar.activation(raw_sb[:, :w], ps_raw[:, :w], AF.Copy)
                sq_sb = wp.tile([128, ICW], BF16, tag="sqsb", bufs=6)
                nc.scalar.activation(sq_sb[:, :w], ps_raw[:, :w], AF.Square)
                ps_ss = psS.tile([HPC, ICW], F32, tag="small")
                nc.tensor.matmul(ps_ss[:, :w], lhsT=hsel[:, :], rhs=sq_sb[:, :w],
                                 start=True, stop=True)
                rec = wp.tile([HPC, ICW], F32, tag="rec", bufs=6)
                nc.vector.reciprocal(rec[:, :w], ps_ss[:, :w])
                rno = wp.tile([HPC, ICW], BF16, tag="rno", bufs=6)
                nc.scalar.activation(rno[:, :w], rec[:, :w], AF.Sqrt,
                                     scale=float(sqrt_scale))
                ps_rep = psS.tile([128, ICW], F32, tag="small")
                nc.tensor.matmul(ps_rep[:, :w], lhsT=hr_row[:, :],
                                 rhs=rno[:, :w], start=True, stop=True)
                nc.vector.tensor_tensor(out_ap, raw_sb[:, :w], ps_rep[:, :w],
                                        op=ALU.mult)

            for b in range(B):
                for c in range(NIC):
                    col0 = b * N + c * ICW
                    # sum(x) row for the mean correction
                    ps_mu = psS.tile([1, ICW], F32, tag="small")
                    for dt in range(NDT):
                        nc.tensor.matmul(ps_mu[:, :], lhsT=ones_col[:, :],
                                         rhs=xts[dt][:, col0:col0 + ICW],
                                         start=(dt == 0), stop=(dt == NDT - 1))
                    mu_row = wp.tile([1, ICW], BF16, tag="murow", bufs=4)
                    nc.vector.tensor_copy(mu_row[:, :], ps_mu[:, :])
                    # Q: raw projection + rank-1 mean correction
                    ps_q = proj_psum(wq_s, col0, close=False)
                    nc.tensor.matmul(ps_q[:, :], lhsT=sqn_s[:, :], rhs=mu_row[:, :],
                                     start=False, stop=True)
                    norm_finish(ps_q, ICW, hrq_s, SCALE * SCALE,
                                qhat[b][:, c * ICW:(c + 1) * ICW])
                    # K
                    ps_k = proj_psum(wk_s, col0, close=True)
                    norm_finish(ps_k, ICW, hrk_s, 1.0,
                                khat[b][:, c * ICW:(c + 1) * ICW])
                    # V: project then PE-transpose into j-major vbuf
                    ps_v = proj_psum(wv_s, col0, close=True)
                    vT_sb = wp.tile([128, ICW], BF16, tag="vtsb", bufs=4)
                    nc.vector.tensor_copy(vT_sb[:, :], ps_v[:, :])
                    for tt in range(ICW // 128):
                        jt = c * (ICW // 128) + tt
                        ps_t = psS.tile([128, 128], BF16, tag="small")
                        nc.tensor.transpose(ps_t[:, :], vT_sb[:, tt * 128:(tt + 1) * 128],
                                            ident[:, :])
                        for h in range(HPC):
                            nc.vector.tensor_copy(
                                vbuf[b][:, jt * VSTRIDE + h * HW:jt * VSTRIDE + h * HW + DH],
                                ps_t[:, h * DH:(h + 1) * DH])

            # null-k normalization (shared across b)
            nksq = wp.tile([128, NNK], BF16, tag="nksq")
            nc.scalar.activation(nksq[:, :], nkT_s[:, :], AF.Square)
            ps_nss = psS.tile([HPC, NNK], F32, tag="small")
            nc.tensor.matmul(ps_nss[:, :], lhsT=hsel[:, :], rhs=nksq[:, :],
                             start=True, stop=True)
            nrec = wp.tile([HPC, NNK], F32, tag="nrec")
            nc.vector.reciprocal(nrec[:, :], ps_nss[:, :])
            nrno = wp.tile([HPC, NNK], BF16, tag="nrno")
            nc.scalar.activation(nrno[:, :], nrec[:, :], AF.Sqrt, scale=1.0)
            ps_nrep = psS.tile([128, NNK], F32, tag="small")
            nc.tensor.matmul(ps_nrep[:, :], lhsT=hrk_s[:, :], rhs=nrno[:, :],
                             start=True, stop=True)
            nc.vector.tensor_tensor(nkhat[:, :], nkT_s[:, :], ps_nrep[:, :], op=ALU.mult)

            # ---------------- attention (ic outer, Wo interleaved) ----------------

            def wo_chunk(b, ic):
                for tt in range(ic * (ICW // 128), (ic + 1) * (ICW // 128)):
                    for cc in range(DIM // ICW):
                        ps_f = psF.tile([128, ICW], F32, tag="pf",
                                        name=f"pf_{b}_{tt}_{cc}")
                        nc.tensor.matmul(ps_f[:, :],
                                         lhsT=outT[b][:, tt * 128:(tt + 1) * 128],
                                         rhs=wo_s[:, cc * ICW:(cc + 1) * ICW],
                                         start=True, stop=True, skip_group_check=True)
                        f_sb = wp.tile([128, ICW], BF16, tag="fsb", bufs=4)
                        if (tt + cc) % 2 == 0:
                            nc.vector.tensor_copy(f_sb[:, :], ps_f[:, :])
                        else:
                            nc.scalar.activation(f_sb[:, :], ps_f[:, :], AF.Copy)
                        nc.sync.dma_start(
                            out=out_d[b, tt * 128:(tt + 1) * 128, cc * ICW:(cc + 1) * ICW],
                            in_=f_sb[:, :])

            for ic in range(NIC):
                i0 = ic * ICW
                njts = (ic + 1) * (ICW // JTW)
                for h in range(HPC):
                    hp = h * DH
                    pos = [poP.tile([HW, ICW], F32, tag="po", name=f"po_{h}_{ic}_{b}") for b in range(B)]
                    for jt in range(njts):
                        bt = bp.tile([128, ICW], BF16, tag="bias")
                        nc.sync.dma_start(
                            out=bt[:, :],
                            in_=biasT_d[h, jt * JTW:(jt + 1) * JTW, i0:i0 + ICW])
                        for b in range(B):
                            ps_s = psB.tile([128, ICW], F32, tag="big")
                            nc.tensor.matmul(
                                ps_s[:, :], lhsT=ident[:, :], rhs=bt[:, :],
                                start=True, stop=False, skip_group_check=True)
                            nc.tensor.matmul(
                                ps_s[:, :],
                                lhsT=khat[b][hp:hp + DH, jt * JTW:(jt + 1) * JTW],
                                rhs=qhat[b][hp:hp + DH, i0:i0 + ICW],
                                start=False, stop=True, skip_group_check=True)
                            e_sb = wp.tile([128, ICW], BF16, tag="esb", bufs=6)
                            nc.scalar.activation(
                                e_sb[:, :], ps_s[:, :], AF.Exp,
                                bias=km_s[:, b * NJT + jt:b * NJT + jt + 1])
                            nc.tensor.matmul(
                                pos[b][:, :],
                                lhsT=vbuf[b][:, jt * VSTRIDE + h * HW:(jt * VSTRIDE + h * HW) + HW],
                                rhs=e_sb[:, :],
                                start=(jt == 0), stop=False, skip_group_check=True)
                    # null kv tile
                    for b in range(B):
                        ps_n = psS.tile([NNK, ICW], F32, tag="small", name=f"psn_{h}_{ic}_{b}")
                        nc.tensor.matmul(ps_n[:, :],
                                         lhsT=nkhat[hp:hp + DH, :],
                                         rhs=qhat[b][hp:hp + DH, i0:i0 + ICW],
                                         start=True, stop=True, skip_group_check=True)
                        en = wp.tile([NNK, ICW], BF16, tag="en", bufs=4)
                        nc.scalar.activation(en[:, :], ps_n[:, :], AF.Exp)
                        nc.tensor.matmul(pos[b][:, :],
                                         lhsT=nvA_s[:, h * HW:(h + 1) * HW],
                                         rhs=en[:, :],
                                         start=False, stop=True, skip_group_check=True)
                    # epilogue: divide by the denominator row
                    for b in range(B):
                        rd32 = wp.tile([1, ICW], F32, tag="rd32", bufs=4)
                        nc.vector.reciprocal(rd32[:, :], pos[b][DH:DH + 1, :])
                        rdb = wp.tile([1, ICW], BF16, tag="rdb", bufs=4)
                        nc.vector.tensor_copy(rdb[:, :], rd32[:, :])
                        ps_r = psS.tile([DH, ICW], F32, tag="small", name=f"psr_{h}_{ic}_{b}")
                        nc.tensor.matmul(ps_r[:, :], lhsT=ones_row[:, :DH], rhs=rdb[:, :],
                                         start=True, stop=True, skip_group_check=True)
                        oc_sb = wp.tile([DH, ICW], BF16, tag="ocsb", bufs=4)
                        nc.vector.tensor_copy(oc_sb[:, :], pos[b][:DH, :])
                        nc.vector.tensor_tensor(
                            outT[b][hp:hp + DH, i0:i0 + ICW],
                            oc_sb[:, :], ps_r[:, :], op=ALU.mult)
                # both heads done for this ic -> project + store these tokens
                for b in range(B):
                    wo_chunk(b, ic)
            all_ps.close()

    nc.compile()
    return nc


def prep_inputs(x, mask, attn_bias, gamma, null_kv, Wq, Wkv, q_scale, k_scale, Wo):
    """Host-side sharding + layout prep. Returns in_maps for 8 cores."""
    x = np.asarray(x, np.float32)
    mask = np.asarray(mask, bool)
    attn_bias = np.asarray(attn_bias, np.float32)
    gamma = np.asarray(gamma, np.float32)
    null_kv = np.asarray(null_kv, np.float32)
    Wq = np.asarray(Wq, np.float32)
    Wkv = np.asarray(Wkv, np.float32)
    q_scale = np.asarray(q_scale, np.float32)
    k_scale = np.asarray(k_scale, np.float32)
    Wo = np.asarray(Wo, np.float32)

    TOK = B * N
    xT = np.ascontiguousarray(x.reshape(TOK, DIM).T).astype(bf)
    WqG = Wq * gamma[:, None]
    qsks = (q_scale * k_scale).astype(np.float32)
    trimask = np.tril(np.ones((N, N), dtype=bool), -1)  # [j, i] True where j > i
    kmf = np.where(mask, 0.0, NEG).astype(np.float32)   # (B, N)
    km = np.ascontiguousarray(
        kmf.reshape(B, NJT, JTW).transpose(2, 0, 1).reshape(JTW, B * NJT))

    def wlayout(w):  # (1024, 128) -> (128, 8*128) device layout
        return np.ascontiguousarray(
            w.reshape(NDT, 128, 128).transpose(1, 0, 2).reshape(128, NDT * 128)
        ).astype(bf)

    in_maps = []
    for c in range(NCORE):
        h0 = HPC * c
        wq_c = WqG[:, h0 * DH:(h0 + HPC) * DH]
        wk_c = Wkv[:, h0 * DH:(h0 + HPC) * DH]
        wv_c = Wkv[:, H * DH + h0 * DH:H * DH + (h0 + HPC) * DH]
        sqn = (-wq_c.sum(axis=0, dtype=np.float64) / DIM).astype(np.float32)[None, :]
        hr = np.zeros((4, 128), np.float32)
        hr[0, :DH] = 1.0
        hr[1, DH:] = 1.0
        hr[2, :DH] = qsks
        hr[3, DH:] = qsks
        nkv = null_kv[h0:h0 + HPC].reshape(HPC, NNK, 2, DH)
        nk = nkv[:, :, 0]   # (HPC, NNK, DH)
        nv = nkv[:, :, 1]
        nkT = np.ascontiguousarray(nk.transpose(0, 2, 1).reshape(HPC * DH, NNK))
        nvA = np.zeros((NNK, VSTRIDE), np.float32)
        for h in range(HPC):
            nvA[:, h * HW:h * HW + DH] = nv[h]
            nvA[:, h * HW + DH] = 1.0
        bT = np.ascontiguousarray(attn_bias[h0:h0 + HPC].transpose(0, 2, 1)).copy()
        bT[:, trimask] = NEG
        in_maps.append({
            "xT": xT,
            "wq": wlayout(wq_c), "wk": wlayout(wk_c), "wv": wlayout(wv_c),
            "wo": Wo[h0 * DH:(h0 + HPC) * DH, :].astype(bf),
            "sqn": sqn.astype(bf), "hr": hr.astype(bf),
            "nkT": nkT.astype(bf), "nvA": nvA.astype(bf),
            "km": km, "biasT": bT.astype(bf),
        })
    return in_maps


def kernel(x, mask, attn_bias, gamma, null_kv, Wq, Wkv, q_scale, k_scale, Wo):
    if "nc" not in _CACHE:
        _CACHE["nc"] = build_nc()
    nc = _CACHE["nc"]
    in_maps = prep_inputs(x, mask, attn_bias, gamma, null_kv, Wq, Wkv,
                          q_scale, k_scale, Wo)
    res = run_bass_kernel_spmd(nc, in_maps, core_ids=list(range(NCORE)))
    acc = np.zeros((B, N, DIM), np.float32)
    for c in range(NCORE):
        acc += np.asarray(res.results[c]["out"], dtype=np.float32)
    return acc
